# revision 44
# baseline (speedup 1.0000x reference)
"""Trainium2 Bass kernel for nn_AttentionFusionModule (dense_transformer).

Data-parallel over batch: B=8 batch elements -> 8 NeuronCores, one attention
block per core.  Per core (C=256, N=64*64=4096, DQK=32):

  q = wq@main + bq          [32, 4096]   (replicated 4x across partition groups)
  k = wk@light + bk         [32, 4096]   (replicated 4x)
  vT = light^T @ wv^T       [4096, 256]  (32 tiles of [128, 256], bf16)
  S^T[m, n] = sum_d k[d,m] q[d,n]        (4x row-tiled matmuls, contraction=32)
  P = exp(S^T)  (no max subtraction: energies are O(1) by construction)
  sums[n] = sum_m P[m, n]   (bf16 pairwise tree on DVE + ones-matmul)
  out[c, n] = (sum_m vT[m,c] P[m,n]) * (1/sums[n]) + main[c,n] + bv[c]

Self-contained: hardcodes all shapes; only needs the container toolchain
(concourse on PYTHONPATH or /opt/trn_rl_repo).
"""

import sys

for _p in ("/opt/trn_rl_repo", "/root/.axon_site/_ro/trn_rl_repo"):
    if _p not in sys.path:
        sys.path.append(_p)

from contextlib import ExitStack

import ml_dtypes
import numpy as np

import bass_rust
import concourse.bass as bass
import concourse.tile as tile
from concourse import mybir
from concourse.bass_utils import run_bass_kernel_spmd

F32 = mybir.dt.float32
BF16 = mybir.dt.bfloat16

C = 256  # channels
N = 4096  # pixels (64*64)
D = 32  # q/k dim
NCH = 8  # n-chunks
CHW = 512  # chunk width (columns of n per chunk)
MT = 32  # m-tiles of 128
MG = 8  # m-groups of 4 tiles


def _split_multi_waits(nc):
    """This container's walrus rejects more than one sync wait per
    instruction; hoist extra waits onto same-engine NOPs placed just before
    the instruction (per-engine streams preserve block order)."""
    k = 0
    for blk in nc.m.functions[0].blocks:
        insts = blk.instructions
        if not any(
            i.sync_info is not None and len(i.sync_info.on_wait) > 1 for i in insts
        ):
            continue
        new = []
        for inst in insts:
            si = inst.sync_info
            if si is not None and len(si.on_wait) > 1:
                waits = list(si.on_wait)
                for w in waits[:-1]:
                    nop = mybir.InstNoOp(name=f"mswait_{k}")
                    k += 1
                    nop.engine = inst.engine
                    nop.sync_info = bass_rust.SyncInfo(on_wait=[w], on_update=[])
                    new.append(nop)
                inst.sync_info = bass_rust.SyncInfo(
                    on_wait=[waits[-1]], on_update=list(si.on_update)
                )
            new.append(inst)
        blk.instructions = new


def build_nc(reps=1, empty=False):
    """reps>1 statically unrolls the whole computation (for HW timing via
    wall-clock slope); empty=True builds just the constants (overhead probe)."""
    nc = bass.Bass("TRN2", target_bir_lowering=False, debug=False, num_devices=8)

    main_d = nc.declare_dram_parameter("main", [C, N], F32, isOutput=False)
    light_d = nc.declare_dram_parameter("light", [C, N], BF16, isOutput=False)
    wqk_d = nc.declare_dram_parameter("wqk", [C, 256], BF16, isOutput=False)
    wvt_d = nc.declare_dram_parameter("wvt", [C, C], BF16, isOutput=False)
    bias_d = nc.declare_dram_parameter("bias", [C, 2], F32, isOutput=False)
    out_d = nc.declare_dram_parameter("out", [C, N], F32, isOutput=True)

    mm = nc.tensor.matmul
    Exp = mybir.ActivationFunctionType.Exp
    Ln = mybir.ActivationFunctionType.Ln
    ADD = mybir.AluOpType.add
    MUL = mybir.AluOpType.mult

    with tile.TileContext(nc) as tc, ExitStack() as ctx:
        pc = ctx.enter_context(tc.tile_pool(name="const", bufs=1))
        p_main = ctx.enter_context(tc.tile_pool(name="main", bufs=1))
        p_qk = ctx.enter_context(tc.tile_pool(name="qk", bufs=1))
        p_vt = ctx.enter_context(tc.tile_pool(name="vt", bufs=1))
        p_exps = ctx.enter_context(tc.tile_pool(name="exps", bufs=2))
        p_tree = ctx.enter_context(tc.tile_pool(name="tree", bufs=1))
        p_stage = ctx.enter_context(tc.tile_pool(name="stage", bufs=4))
        p_small = ctx.enter_context(tc.tile_pool(name="small", bufs=2))
        ps_s = ctx.enter_context(tc.tile_pool(name="ps_s", bufs=2, space="PSUM"))
        ps_o = ctx.enter_context(tc.tile_pool(name="ps_o", bufs=3, space="PSUM"))
        ps_sum = ctx.enter_context(tc.tile_pool(name="ps_sum", bufs=1, space="PSUM"))

        # ---- constants (packed into few DMAs: HWDGE issue is ~0.6us each) ----
        wqt = []
        wkt = []
        wvt = []
        bvt = []
        biast = []
        for ci in range(2):
            t = pc.tile([128, 256], BF16, tag=f"wqk{ci}", name=f"wqk{ci}")
            nc.sync.dma_start(out=t[:], in_=wqk_d[128 * ci : 128 * (ci + 1), :])
            wqt.append(t[:, 0:128])
            wkt.append(t[:, 128:256])
        for ci in range(2):
            t = pc.tile([128, C], BF16, tag=f"wvt{ci}", name=f"wvt{ci}")
            wvt.append(t)
            t = pc.tile([128, 2], F32, tag=f"bias{ci}", name=f"biast{ci}")
            biast.append(t)
            bvt.append(t[:, 1:2])
        bqr = biast[0][:, 0:1]
        bkr = biast[1][:, 0:1]
        ones_bf = pc.tile([128, 1], BF16, tag="ones", name="ones_bf")
        nc.vector.memset(ones_bf[:], 1.0)
        ones_row = pc.tile([1, 128], F32, tag="ones_row", name="ones_row")
        nc.vector.memset(ones_row[:], 1.0)
        rscratch = nc.dram_tensor("rscratch", [1, CHW], F32)

        def emit(r):
            # ---- phase 1: loads + projections ----
            # light_bf shares the expS pool slots (phase-1-only lifetime);
            # main_bf shares the tree-temp slots (tA/tB used from phase 2 on).
            # First 512-col slices land fast so chunk-0 projections start
            # early; weight/bias DMAs for later phases are issued after them.
            main_f = []
            main_bf = []
            light_bf = []
            for ci in range(2):
                t = p_main.tile([128, N], F32, tag=f"main{ci}", name=f"r{r}main{ci}")
                main_f.append(t)
                lt = p_exps.tile([128, N], BF16, tag="expS", name=f"r{r}light_bf{ci}")
                light_bf.append(lt)
                mt_ = p_tree.tile(
                    [128, N],
                    BF16,
                    tag=("tA" if ci == 0 else "tB"),
                    name=f"r{r}main_bf{ci}",
                )
                main_bf.append(mt_)
            slices = [slice(0, 512), slice(512, 2048), slice(2048, 4096)]
            for j, csl in enumerate(slices):
                for ci in range(2):
                    rsl = slice(128 * ci, 128 * (ci + 1))
                    nc.sync.dma_start(out=main_f[ci][:, csl], in_=main_d[rsl, csl])
                    nc.scalar.dma_start(out=light_bf[ci][:, csl], in_=light_d[rsl, csl])
                    nc.vector.tensor_copy(main_bf[ci][:, csl], main_f[ci][:, csl])
                if r == 0 and j == 0:
                    # queue the remaining constants behind the first slices
                    for ci in range(2):
                        nc.scalar.dma_start(
                            out=wvt[ci][:], in_=wvt_d[128 * ci : 128 * (ci + 1), :]
                        )
                        nc.scalar.dma_start(
                            out=biast[ci][:], in_=bias_d[128 * ci : 128 * (ci + 1), :]
                        )

            q_rep = p_qk.tile([128, N], BF16, tag="q_rep", name=f"r{r}q_rep")
            k_rep = p_qk.tile([128, N], BF16, tag="k_rep", name=f"r{r}k_rep")
            for ch in range(NCH):
                sl = slice(CHW * ch, CHW * (ch + 1))
                pq = ps_o.tile([128, CHW], F32, tag="o", name=f"r{r}pq{ch}")
                mm(pq[:], wqt[0], main_bf[0][:, sl], start=True, stop=False)
                mm(pq[:], wqt[1], main_bf[1][:, sl], start=False, stop=True)
                nc.vector.tensor_scalar_add(q_rep[:, sl], pq[:], bqr)
                pk = ps_o.tile([128, CHW], F32, tag="o", name=f"r{r}pk{ch}")
                mm(pk[:], wkt[0], light_bf[0][:, sl], start=True, stop=False)
                mm(pk[:], wkt[1], light_bf[1][:, sl], start=False, stop=True)
                nc.vector.tensor_scalar_add(k_rep[:, sl], pk[:], bkr)

            # vT tiles: [m-within-tile, mt, c]
            vt_sb = p_vt.tile([128, MT, C], BF16, tag="vt", name=f"r{r}vt")
            for nt in range(MT):
                sl = slice(128 * nt, 128 * (nt + 1))
                pv = ps_o.tile([128, C], F32, tag="o", name=f"r{r}pv{nt}")
                mm(pv[:], light_bf[0][:, sl], wvt[0][:], start=True, stop=False)
                mm(pv[:], light_bf[1][:, sl], wvt[1][:], start=False, stop=True)
                nc.vector.tensor_copy(vt_sb[:, nt, :], pv[:])

            # ---- phase 2: attention main loop over n-chunks ----
            tA = p_tree.tile([128, 8192], BF16, tag="tA", name=f"r{r}tA")
            tB = p_tree.tile([128, 4096], BF16, tag="tB", name=f"r{r}tB")

            for ch in range(NCH):
                last = ch == NCH - 1
                nsl = slice(CHW * ch, CHW * (ch + 1))
                expS = p_exps.tile([128, MT * CHW], BF16, tag="expS", name=f"r{r}eS{ch}")
                sums_p = p_small.tile(
                    [128, CHW], BF16, tag="sums_p", name=f"r{r}sp{ch}"
                )

                # QK^T + exp: groups of 2 m-tiles, double-buffered psum so the
                # next group's matmuls overlap this group's exp
                for g in range(2 * MG):
                    ps = ps_s.tile([128, 2, CHW], F32, tag="s", name=f"r{r}ps{ch}_{g}")
                    for i in range(2):
                        mt = 2 * g + i
                        p0 = 32 * i
                        mm(
                            ps[:, i, :],
                            k_rep[p0 : p0 + 32, 128 * mt : 128 * (mt + 1)],
                            q_rep[p0 : p0 + 32, nsl],
                            start=True,
                            stop=True,
                            tile_position=(p0, 0),
                            skip_group_check=True,
                        )
                    nc.scalar.activation(
                        expS[:, 1024 * g : 1024 * (g + 1)], ps[:, :, :], Exp
                    )
                    if last:
                        # accumulate the softmax denominator per group so the
                        # kernel tail doesn't wait on a post-hoc tree
                        base = 1024 * g
                        if g == 0:
                            nc.vector.tensor_tensor(
                                sums_p[:], expS[:, 0:512], expS[:, 512:1024], ADD
                            )
                        else:
                            gt = p_small.tile(
                                [128, CHW], BF16, tag="gt", name=f"r{r}gt{ch}_{g}"
                            )
                            nc.vector.tensor_tensor(
                                gt[:],
                                expS[:, base : base + 512],
                                expS[:, base + 512 : base + 1024],
                                ADD,
                            )
                            nc.vector.tensor_tensor(sums_p[:], sums_p[:], gt[:], ADD)

                po = [
                    ps_o.tile([128, CHW], F32, tag="o", name=f"r{r}po{ch}_{cj}")
                    for cj in range(2)
                ]
                unnorm = (
                    []
                    if last
                    else [
                        p_stage.tile(
                            [128, CHW], BF16, tag="unnorm", name=f"r{r}un{ch}_{cj}"
                        )
                        for cj in range(2)
                    ]
                )
                recip_bc = p_small.tile(
                    [128, CHW], F32, tag="recip_bc", name=f"r{r}rbc{ch}"
                )

                def av_stream(cj):
                    for mt in range(MT):
                        mm(
                            po[cj][:],
                            vt_sb[:, mt, 128 * cj : 128 * (cj + 1)],
                            expS[:, CHW * mt : CHW * (mt + 1)],
                            start=(mt == 0),
                            stop=(mt == MT - 1),
                            skip_group_check=True,
                        )
                    nc.vector.tensor_copy(unnorm[cj][:], po[cj][:])

                def recip_chain():
                    # reduce 128 partitions -> [1,512] on PE; 1/x = exp(-ln)
                    psm = ps_sum.tile([1, CHW], F32, tag="sums", name=f"r{r}psm{ch}")
                    mm(psm[:], ones_bf[:], sums_p[:], start=True, stop=True)
                    recip1 = p_small.tile(
                        [1, CHW], F32, tag="recip1", name=f"r{r}rc{ch}"
                    )
                    nc.scalar.activation(recip1[:], psm[:], Ln)
                    nc.scalar.activation(recip1[:], recip1[:], Exp, scale=-1.0)
                    if last:
                        # broadcast via ones-matmul (PE has slack at the tail);
                        # evacuate on ACT to keep DVE's FIFO clear
                        pbc = ps_o.tile([128, CHW], F32, tag="o", name=f"r{r}pbc{ch}")
                        mm(pbc[:], ones_row[:], recip1[:], start=True, stop=True)
                        nc.scalar.copy(recip_bc[:], pbc[:])
                    else:
                        # broadcast [1,512]->[128,512]: bounce via DRAM, then
                        # a stride-0-partition DMA read (DRAM sources only)
                        nc.gpsimd.dma_start(out=rscratch[:, :], in_=recip1[:])
                        rap = rscratch.ap()
                        bcast_src = bass.AP(
                            tensor=rap.tensor,
                            offset=rap.offset,
                            ap=[[0, 128], rap.ap[1]],
                        )
                        nc.gpsimd.dma_start(out=recip_bc[:], in_=bcast_src)

                def epilogue(cj):
                    # out = unnorm * recip + (main + bv), then DMA out
                    tmp = p_stage.tile(
                        [128, CHW], F32, tag="tmp", name=f"r{r}tm{ch}{cj}"
                    )
                    nc.vector.tensor_tensor(tmp[:], unnorm[cj][:], recip_bc[:], MUL)
                    stg = p_stage.tile(
                        [128, CHW], F32, tag="stg", name=f"r{r}sg{ch}{cj}"
                    )
                    nc.vector.scalar_tensor_tensor(
                        stg[:], main_f[cj][:, nsl], bvt[cj], tmp[:], ADD, ADD
                    )
                    eng = nc.sync if cj == 0 else nc.scalar
                    eng.dma_start(
                        out=out_d[128 * cj : 128 * (cj + 1), nsl], in_=stg[:]
                    )

                if last:
                    for mt in range(MT):
                        esl = expS[:, CHW * mt : CHW * (mt + 1)]
                        for cj in range(2):
                            mm(
                                po[cj][:],
                                vt_sb[:, mt, 128 * cj : 128 * (cj + 1)],
                                esl,
                                start=(mt == 0),
                                stop=(mt == MT - 1),
                                skip_group_check=True,
                            )
                        if mt == MT - 2:
                            # sums_p complete by now; squeeze the recip chain's
                            # PE ops in before the final AV matmuls
                            recip_chain()
                    for cj in range(2):
                        # tail: normalize straight out of PSUM (no unnorm hop)
                        tmp = p_stage.tile(
                            [128, CHW], F32, tag="tmp", name=f"r{r}tm{ch}{cj}"
                        )
                        nc.vector.tensor_tensor(tmp[:], po[cj][:], recip_bc[:], MUL)
                        stg = p_stage.tile(
                            [128, CHW], F32, tag="stg", name=f"r{r}sg{ch}{cj}"
                        )
                        nc.vector.scalar_tensor_tensor(
                            stg[:], main_f[cj][:, nsl], bvt[cj], tmp[:], ADD, ADD
                        )
                        eng = nc.sync if cj == 0 else nc.scalar
                        eng.dma_start(
                            out=out_d[128 * cj : 128 * (cj + 1), nsl], in_=stg[:]
                        )
                else:
                    for mt in range(MT):
                        esl = expS[:, CHW * mt : CHW * (mt + 1)]
                        for cj in range(2):
                            mm(
                                po[cj][:],
                                vt_sb[:, mt, 128 * cj : 128 * (cj + 1)],
                                esl,
                                start=(mt == 0),
                                stop=(mt == MT - 1),
                                skip_group_check=True,
                            )
                    for cj in range(2):
                        nc.vector.tensor_copy(unnorm[cj][:], po[cj][:])
                    # softmax denominator: bf16 pairwise tree over the 32
                    # m-tiles (L1 split so DVE's FIFO can interleave copies)
                    nc.vector.tensor_tensor(
                        tA[:, 0:4096], expS[:, 0:4096], expS[:, 8192:12288], ADD
                    )
                    nc.vector.tensor_tensor(
                        tA[:, 4096:8192], expS[:, 4096:8192], expS[:, 12288:16384], ADD
                    )
                    nc.vector.tensor_tensor(
                        tB[:, :], tA[:, 0:4096], tA[:, 4096:8192], ADD
                    )
                    nc.vector.tensor_tensor(
                        tA[:, 0:2048], tB[:, 0:2048], tB[:, 2048:4096], ADD
                    )
                    nc.vector.tensor_tensor(
                        tB[:, 0:1024], tA[:, 0:1024], tA[:, 1024:2048], ADD
                    )
                    nc.vector.tensor_tensor(
                        sums_p[:], tB[:, 0:512], tB[:, 512:1024], ADD
                    )
                    recip_chain()
                    epilogue(0)
                    epilogue(1)

        if not empty:
            for r in range(reps):
                emit(r)

    _split_multi_waits(nc)
    return nc


_NC_CACHE = {}


def _get_nc():
    if "nc" not in _NC_CACHE:
        _NC_CACHE["nc"] = build_nc()
    return _NC_CACHE["nc"]


def kernel(main_feature, light_feature, wq, bq, wk, bk, wv, bv):
    B, Cc, H, W = main_feature.shape
    assert (B, Cc, H * W) == (8, C, N), (B, Cc, H, W)
    bf = ml_dtypes.bfloat16

    main = np.ascontiguousarray(main_feature.reshape(B, C, N), dtype=np.float32)
    light = np.ascontiguousarray(light_feature.reshape(B, C, N)).astype(bf)
    wqt = np.concatenate([np.asarray(wq).T] * 4, axis=1)
    wkt = np.concatenate([np.asarray(wk).T] * 4, axis=1)
    wqk = np.ascontiguousarray(np.concatenate([wqt, wkt], axis=1)).astype(bf)
    wvt = np.ascontiguousarray(np.asarray(wv).T).astype(bf)
    bqr = np.tile(np.asarray(bq, np.float32), 4)
    bkr = np.tile(np.asarray(bk, np.float32), 4)
    bias = np.zeros((C, 2), np.float32)
    bias[0:128, 0] = bqr
    bias[128:256, 0] = bkr
    bias[:, 1] = np.asarray(bv, np.float32)
    bias = np.ascontiguousarray(bias)

    nc = _get_nc()
    in_maps = [
        {
            "main": main[b],
            "light": light[b],
            "wqk": wqk,
            "wvt": wvt,
            "bias": bias,
        }
        for b in range(B)
    ]
    res = run_bass_kernel_spmd(nc, in_maps, core_ids=list(range(8)), trace=False)
    out = np.stack([res.results[b]["out"] for b in range(B)], axis=0)
    return out.reshape(B, C, H, W).astype(np.float32)


if __name__ == "__main__":
    nc = build_nc()
    print(
        "built OK; instructions:",
        sum(len(b.instructions) for b in nc.m.functions[0].blocks),
    )
